# revision 10
# baseline (speedup 1.0000x reference)
"""Dilated attention TRN2 kernel (nn_DilatedAttention_5634997092450).

Math (reference.py): x [4, 8192, 1024] fp32. Per 1024-token segment keep
every 2nd token (offset 0) -> 512 tokens. 16-head self-attention (q=k=v)
within each sparse segment, head_dim 64, scale 1/8. The reference's
denominator-weighted scatter-add merge is the identity (each global
position is hit exactly once), so the output is just the attention output
scattered to even rows; odd rows are zero.

Device strategy (8 NeuronCores, data-parallel over the 32 (batch, segment)
units, 4 per core):
  - QK^T as lhsT.T @ rhs with lhsT = Q^T [64, 128] slices of xsT, rhs =
    K^T [64, 512]; float32r dtype (fp32 data at ~bf16 speed, ~1.6e-4 err).
    Even/odd heads sit on PE row-groups 0-63/64-127 -> concurrent MMs.
  - softmax without max subtraction (scaled scores <= ~14): one ScalarE
    exp pass per 2 PSUM banks with the 1/8 scale fused into the
    activation's affine prelude. Scores are symmetric (q=k), so the
    exp'd tile serves directly as the PV moving operand.
  - PV with V_aug = [V | ones] stationary [128, 65]: PSUM row 64
    accumulates the softmax denominators for free.
  - [65, 512] unnormalized result DMA'd PSUM -> DRAM; the host divides by
    the denominator row, transposes, and scatters into even output rows.
"""

import numpy as np

import concourse.bass as bass
import concourse.mybir as mybir
from concourse.tile import TileContext


N_CORES = 8
B, N, D = 4, 8192, 1024
NUM_HEADS = 16
HEAD_DIM = 64
SEGMENT = 1024
DILATION = 2
K = SEGMENT // DILATION          # 512 kept tokens per segment
N_SEG = N // SEGMENT             # 8
UNITS_PER_CORE = (B * N_SEG) // N_CORES  # 4
SCALE = 1.0 / 8.0                # 1/sqrt(HEAD_DIM)

F32 = mybir.dt.float32
F32R = mybir.dt.float32r


def _split_multi_waits(nc):
    # This walrus build rejects >1 sem-wait per instruction ("Too many
    # sync wait commands"). Hoist all but one wait of any multi-wait
    # instruction onto single-wait same-engine nops inserted just before
    # it; engines execute their stream in order, so semantics hold.
    for f in nc.m.functions:
        for bb in f.blocks:
            out = []
            for inst in bb.instructions:
                si = inst.sync_info
                if si is not None and len(si.on_wait) > 1:
                    waits = list(si.on_wait)
                    for w in waits[:-1]:
                        nop = mybir.InstNoOp(
                            name=nc.get_next_instruction_name(),
                            ins=[], outs=[], engine=inst.engine)
                        nop.sync_info = mybir.SyncInfo(on_wait=[w], on_update=[])
                        out.append(nop)
                    inst.sync_info = mybir.SyncInfo(
                        on_wait=[waits[-1]], on_update=list(si.on_update))
                out.append(inst)
            bb.instructions = out


def build_kernel(units=UNITS_PER_CORE, heads=NUM_HEADS, split_waits=True):
    nc = bass.Bass()
    xsT = nc.dram_tensor("xsT", [units, D, K], F32R, kind="ExternalInput")
    xs = nc.dram_tensor("xs", [units, K, D], F32R, kind="ExternalInput")
    outTd = nc.dram_tensor("outTd", [units, heads, HEAD_DIM + 1, K], F32,
                           kind="ExternalOutput")

    n_ktile = K // 128            # 4
    n_qtile = K // 128            # 4

    with TileContext(nc) as tc:
        with tc.tile_pool(name="inp", bufs=2) as inp_pool, \
             tc.tile_pool(name="upool", bufs=3) as u_pool, \
             tc.tile_pool(name="ops", bufs=4) as o_pool, \
             tc.tile_pool(name="spool", bufs=1, space="PSUM") as s_pool:
            # Hand-managed PSUM: all 8 banks in one tensor. Head h uses
            # bank group g = h%2 (banks 4g..4g+3) for its scores; its PV
            # output accumulates into bank 4g+3 after exp frees the group.
            # Tile's bank-overlap tracking serializes same-bank reuse.
            ps = s_pool.tile([128, 8, K], F32, tag="ps")
            ones_sb = o_pool.tile([128, n_ktile * heads], F32, tag="ones")
            nc.vector.memset(ones_sb, 1.0)
            ones_r = ones_sb.rearrange("p (j h) -> p j h", j=n_ktile)
            for u in range(units):
                # xsT: [128, d_chunk, tok]; chunk c holds d in [128c, 128c+128)
                xsT_sb = inp_pool.tile([128, D // 128, K], F32R, tag="xsT")
                nc.sync.dma_start(out=xsT_sb, in_=xsT[u].rearrange("(c p) t -> p c t", p=128))
                # V_aug: [128, ktile, head, 65]; cols 0:64 = V, col 64 = ones
                vaug_sb = inp_pool.tile([128, n_ktile, heads, HEAD_DIM + 1], F32R, tag="vaug")
                xs_r = xs[u].rearrange("(j p) (h e) -> j p h e", p=128, e=HEAD_DIM)
                for j in range(n_ktile):
                    nc.sync.dma_start(out=vaug_sb[:, j, :, 0:HEAD_DIM],
                                      in_=xs_r[j, :, 0:heads, :])
                nc.vector.tensor_copy(
                    vaug_sb[:, :, :, HEAD_DIM:HEAD_DIM + 1].rearrange("p j h one -> p (j h one)"),
                    ones_r[:, :, 0:heads].rearrange("p j h -> p (j h)"))

                for h in range(heads):
                    chunk = h // 2
                    plo = 64 * (h % 2)          # 0 or 64: PE row group
                    phi = plo + HEAD_DIM
                    g4 = 4 * (h % 2)            # PSUM bank group base
                    u_sb = u_pool.tile([128, n_ktile, K], F32R, tag="u")
                    for i in range(n_qtile):
                        nc.tensor.matmul(
                            ps[:, g4 + i, :],
                            xsT_sb[plo:phi, chunk, 128 * i:128 * (i + 1)],
                            xsT_sb[plo:phi, chunk, :],
                            start=True, stop=True,
                        )
                    # exp(S/8): [128, 2048] in one ACT instruction
                    nc.scalar.activation(
                        u_sb, ps[:, g4:g4 + 4, :],
                        mybir.ActivationFunctionType.Exp, scale=SCALE,
                    )
                    pv = ps[0:HEAD_DIM + 1, g4 + 3, :]
                    for j in range(n_ktile):
                        nc.tensor.matmul(
                            pv,
                            vaug_sb[:, j, h, :],
                            u_sb[:, j, :],
                            start=(j == 0), stop=(j == n_ktile - 1),
                        )
                    o_sb = o_pool.tile([HEAD_DIM + 1, K], F32, tag="ostage")
                    nc.vector.tensor_copy(o_sb, pv)
                    nc.sync.dma_start(out=outTd[u, h], in_=o_sb)
    if split_waits:
        _split_multi_waits(nc)
    return nc


def kernel(x: np.ndarray) -> np.ndarray:
    from concourse.bass_utils import run_bass_kernel_spmd

    b, n, d = x.shape
    assert (b, n, d) == (B, N, D)
    x = np.ascontiguousarray(x, dtype=np.float32)
    # dilated gather: every 2nd token of each segment
    xs_all = x.reshape(B, N_SEG, SEGMENT, D)[:, :, 0::DILATION, :]  # [B, S, K, D]

    nc = build_kernel()
    in_maps = []
    for c in range(N_CORES):
        us = [(uu // N_SEG, uu % N_SEG) for uu in range(UNITS_PER_CORE * c,
                                                        UNITS_PER_CORE * (c + 1))]
        xs_c = np.ascontiguousarray(
            np.stack([xs_all[bb, ss] for bb, ss in us]))            # [U, K, D]
        xsT_c = np.ascontiguousarray(xs_c.transpose(0, 2, 1))        # [U, D, K]
        in_maps.append({"xs": xs_c, "xsT": xsT_c})

    res = run_bass_kernel_spmd(nc, in_maps, core_ids=list(range(N_CORES)))

    out = np.zeros((B, N, D), dtype=np.float32)
    idx = np.arange(0, SEGMENT, DILATION)
    for c in range(N_CORES):
        o = res.results[c]["outTd"]  # [U, heads, 65, K]
        for ui in range(UNITS_PER_CORE):
            uu = UNITS_PER_CORE * c + ui
            bb, ss = uu // N_SEG, uu % N_SEG
            vals = o[ui, :, 0:HEAD_DIM, :] / o[ui, :, HEAD_DIM:HEAD_DIM + 1, :]
            # vals [heads, 64, K] -> [K, heads*64]
            seg = vals.transpose(2, 0, 1).reshape(K, D)
            out[bb, ss * SEGMENT + idx, :] = seg
    return out


# revision 13
# speedup vs baseline: 1.0093x; 1.0093x over previous
"""Dilated attention TRN2 kernel (nn_DilatedAttention_5634997092450).

Math (reference.py): x [4, 8192, 1024] fp32. Per 1024-token segment keep
every 2nd token (offset 0) -> 512 tokens. 16-head self-attention (q=k=v)
within each sparse segment, head_dim 64, scale 1/8. The reference's
denominator-weighted scatter-add merge is the identity (each global
position is hit exactly once), so the output is just the attention output
scattered to even rows; odd rows are zero.

Device strategy (8 NeuronCores, data-parallel over the 32 (batch, segment)
units, 4 per core):
  - QK^T as lhsT.T @ rhs with lhsT = Q^T [64, 128] slices of xsT, rhs =
    K^T [64, 512]; float32r dtype (fp32 data at ~bf16 speed, ~1.6e-4 err).
    Even/odd heads sit on PE row-groups 0-63/64-127 -> concurrent MMs.
  - softmax without max subtraction (scaled scores <= ~14): one ScalarE
    exp pass per 2 PSUM banks with the 1/8 scale fused into the
    activation's affine prelude. Scores are symmetric (q=k), so the
    exp'd tile serves directly as the PV moving operand.
  - PV with V_aug = [V | ones] stationary [128, 65]: PSUM row 64
    accumulates the softmax denominators for free.
  - [65, 512] unnormalized result DMA'd PSUM -> DRAM; the host divides by
    the denominator row, transposes, and scatters into even output rows.
"""

import numpy as np

import concourse.bass as bass
import concourse.mybir as mybir
from concourse.tile import TileContext


N_CORES = 8
B, N, D = 4, 8192, 1024
NUM_HEADS = 16
HEAD_DIM = 64
SEGMENT = 1024
DILATION = 2
K = SEGMENT // DILATION          # 512 kept tokens per segment
N_SEG = N // SEGMENT             # 8
UNITS_PER_CORE = (B * N_SEG) // N_CORES  # 4
SCALE = 1.0 / 8.0                # 1/sqrt(HEAD_DIM)

F32 = mybir.dt.float32
F32R = mybir.dt.float32r


def _split_multi_waits(nc):
    # This walrus build rejects >1 sem-wait per instruction ("Too many
    # sync wait commands"). Hoist all but one wait of any multi-wait
    # instruction onto single-wait same-engine nops inserted just before
    # it; engines execute their stream in order, so semantics hold.
    for f in nc.m.functions:
        for bb in f.blocks:
            out = []
            for inst in bb.instructions:
                si = inst.sync_info
                if si is not None and len(si.on_wait) > 1:
                    waits = list(si.on_wait)
                    for w in waits[:-1]:
                        nop = mybir.InstNoOp(
                            name=nc.get_next_instruction_name(),
                            ins=[], outs=[], engine=inst.engine)
                        nop.sync_info = mybir.SyncInfo(on_wait=[w], on_update=[])
                        out.append(nop)
                    inst.sync_info = mybir.SyncInfo(
                        on_wait=[waits[-1]], on_update=list(si.on_update))
                out.append(inst)
            bb.instructions = out


def build_kernel(units=UNITS_PER_CORE, heads=NUM_HEADS, split_waits=True, no_exp=False, no_pv=False):
    nc = bass.Bass()
    xsT = nc.dram_tensor("xsT", [units, D, K], F32R, kind="ExternalInput")
    xs = nc.dram_tensor("xs", [units, K, D], F32R, kind="ExternalInput")
    outTd = nc.dram_tensor("outTd", [units, heads, HEAD_DIM + 1, K], F32,
                           kind="ExternalOutput")

    n_ktile = K // 128            # 4
    n_qtile = K // 128            # 4

    with TileContext(nc) as tc:
        with tc.tile_pool(name="inp", bufs=2) as inp_pool, \
             tc.tile_pool(name="upool", bufs=3) as u_pool, \
             tc.tile_pool(name="ops", bufs=4) as o_pool, \
             tc.tile_pool(name="spool", bufs=1, space="PSUM") as s_pool:
            # Hand-managed PSUM: all 8 banks in one tensor. Head h uses
            # bank group g = h%2 (banks 4g..4g+3) for its scores; its PV
            # output accumulates into bank 4g+3 after exp frees the group.
            # Tile's bank-overlap tracking serializes same-bank reuse.
            ps = s_pool.tile([128, 8, K], F32, tag="ps")
            ones_sb = o_pool.tile([128, n_ktile * heads], F32, tag="ones")
            nc.vector.memset(ones_sb, 1.0)
            ones_r = ones_sb.rearrange("p (j h) -> p j h", j=n_ktile)
            for u in range(units):
                # xsT: [128, d_chunk, tok]; chunk c holds d in [128c, 128c+128)
                xsT_sb = inp_pool.tile([128, D // 128, K], F32R, tag="xsT")
                nc.sync.dma_start(out=xsT_sb, in_=xsT[u].rearrange("(c p) t -> p c t", p=128))
                # V_aug: [128, ktile, head, 65]; cols 0:64 = V, col 64 = ones
                vaug_sb = inp_pool.tile([128, n_ktile, heads, HEAD_DIM + 1], F32R, tag="vaug")
                xs_r = xs[u].rearrange("(j p) (h e) -> j p h e", p=128, e=HEAD_DIM)
                for j in range(n_ktile):
                    nc.sync.dma_start(out=vaug_sb[:, j, :, 0:HEAD_DIM],
                                      in_=xs_r[j, :, 0:heads, :])
                nc.vector.tensor_copy(
                    vaug_sb[:, :, :, HEAD_DIM:HEAD_DIM + 1].rearrange("p j h one -> p (j h one)"),
                    ones_r[:, :, 0:heads].rearrange("p j h -> p (j h)"))

                for h in range(heads):
                    chunk = h // 2
                    plo = 64 * (h % 2)          # 0 or 64: PE row group
                    phi = plo + HEAD_DIM
                    g4 = 4 * (h % 2)            # PSUM bank group base
                    u_sb = u_pool.tile([128, n_ktile, K], F32R, tag="u")
                    # qtile i -> bank g4 + (3 - i): bank g4+0 is written LAST,
                    # so the previous same-group head's PV output (parked in
                    # bank g4+0) blocks only this head's final QK^T matmul.
                    for i in range(n_qtile):
                        nc.tensor.matmul(
                            ps[:, g4 + 3 - i, :],
                            xsT_sb[plo:phi, chunk, 128 * i:128 * (i + 1)],
                            xsT_sb[plo:phi, chunk, :],
                            start=True, stop=True,
                        )
                    # exp(S/8): [128, 2048] in one ACT instruction.
                    # u_sb[:, jj, :] = exp(S[qtile 3-jj, :]); by symmetry of S
                    # this tile is also U^T[ktile 3-jj, all q].
                    if no_exp:
                        nc.vector.tensor_copy(u_sb, ps[:, g4:g4 + 4, :])
                    else:
                        nc.scalar.activation(
                            u_sb, ps[:, g4:g4 + 4, :],
                            mybir.ActivationFunctionType.Exp, scale=SCALE,
                        )
                    pv = ps[0:HEAD_DIM + 1, g4 + 0, :]
                    for j in range(0 if no_pv else n_ktile):
                        nc.tensor.matmul(
                            pv,
                            vaug_sb[:, j, h, :],
                            u_sb[:, 3 - j, :],
                            start=(j == 0), stop=(j == n_ktile - 1),
                        )
                    o_sb = o_pool.tile([HEAD_DIM + 1, K], F32, tag="ostage")
                    if no_pv:
                        nc.vector.tensor_copy(o_sb, ps[0:HEAD_DIM + 1, g4 + 3, :])
                    else:
                        nc.vector.tensor_copy(o_sb, pv)
                    nc.sync.dma_start(out=outTd[u, h], in_=o_sb)
    if split_waits:
        _split_multi_waits(nc)
    return nc


def kernel(x: np.ndarray) -> np.ndarray:
    from concourse.bass_utils import run_bass_kernel_spmd

    b, n, d = x.shape
    assert (b, n, d) == (B, N, D)
    x = np.ascontiguousarray(x, dtype=np.float32)
    # dilated gather: every 2nd token of each segment
    xs_all = x.reshape(B, N_SEG, SEGMENT, D)[:, :, 0::DILATION, :]  # [B, S, K, D]

    nc = build_kernel()
    in_maps = []
    for c in range(N_CORES):
        us = [(uu // N_SEG, uu % N_SEG) for uu in range(UNITS_PER_CORE * c,
                                                        UNITS_PER_CORE * (c + 1))]
        xs_c = np.ascontiguousarray(
            np.stack([xs_all[bb, ss] for bb, ss in us]))            # [U, K, D]
        xsT_c = np.ascontiguousarray(xs_c.transpose(0, 2, 1))        # [U, D, K]
        in_maps.append({"xs": xs_c, "xsT": xsT_c})

    res = run_bass_kernel_spmd(nc, in_maps, core_ids=list(range(N_CORES)))

    out = np.zeros((B, N, D), dtype=np.float32)
    idx = np.arange(0, SEGMENT, DILATION)
    for c in range(N_CORES):
        o = res.results[c]["outTd"]  # [U, heads, 65, K]
        for ui in range(UNITS_PER_CORE):
            uu = UNITS_PER_CORE * c + ui
            bb, ss = uu // N_SEG, uu % N_SEG
            vals = o[ui, :, 0:HEAD_DIM, :] / o[ui, :, HEAD_DIM:HEAD_DIM + 1, :]
            # vals [heads, 64, K] -> [K, heads*64]
            seg = vals.transpose(2, 0, 1).reshape(K, D)
            out[bb, ss * SEGMENT + idx, :] = seg
    return out
